# revision 12
# baseline (speedup 1.0000x reference)
"""GRU cell kernel for Trainium2, data-parallel over 8 NeuronCores.

Reference computation (B=4096, I=H=2048, C=I+H=4096):
    combined   = [x, h]                                   [B, C]
    to_update  = sigmoid(combined @ W_update.T + b_u)     [B, H]
    to_select  = sigmoid(combined @ W_select.T + b_s)     [B, H]
    updated    = h * to_update
    new_comb   = [x, updated]
    predictions= tanh(new_comb @ W_predict.T + b_p)
    h_new      = h * (1 - to_select) + predictions * to_select

Sharding: batch split 8 ways (512 rows/core), weights replicated.
On-chip layout is [feature, batch] (transposed), so each weight tile
[128c, 128h] is the stationary matmul operand and activation tiles
[128c, 512b] are the moving operand -- no on-chip transposes anywhere.
Matmuls run in bf16 (inputs host-cast) with fp32 PSUM accumulation;
the final blend uses fp32 h.

Latency structure (per core):
  * 1536 matmuls x ~215 ns is the PE floor (~332 us incl. overhead);
    everything else is arranged to keep PE busy from ~3.5 us onward.
  * DMA descriptor processing costs ~0.6 us per DMA regardless of size,
    so x / h / biases are uploaded as a few 0.5-1 MB chunk DMAs into
    flat [128, n*512] SBUF tiles; per-c-tile views feed the matmuls
    (subtile deps let each matmul wait only on its own chunk).
  * h is uploaded twice (bf16 for matmuls, fp32 for the blend) so no
    on-chip cast sits on the critical path; the fp32 copy is issued
    after the update phase (only the blend needs it).
  * The first PRE update-gate gemms run their x-half contraction first,
    ordered chunk-by-chunk across PRE PSUM banks in step with the
    interleaved [wx0 xc0 wx1 xc1 ...] upload order, so the PE starts
    after ~1 MB of DMA and stays ahead of the h upload.
  * h*(1-sel) is precomputed into the fp32 h tile during the select
    phase (DVE slack), so the predict-phase tail per tile is
    tanh + 2 DVE ops + store; the last output tile is split into two
    half-batch PSUM groups to halve the exposed tail.
"""

from contextlib import ExitStack

import numpy as np
import ml_dtypes

import concourse.bass as bass
import concourse.tile as tile
import concourse.mybir as mybir
from concourse import bacc
from concourse.bass_utils import run_bass_kernel_spmd

BF16 = mybir.dt.bfloat16
F32 = mybir.dt.float32
NPBF16 = ml_dtypes.bfloat16

B, I, H = 4096, 2048, 2048
C = I + H
NCORES = 8
BS = B // NCORES            # 512 batch rows per core
P = 128                     # SBUF partitions
HT = H // P                 # 16 output-row tiles
IT = I // P                 # 16 x feature tiles
CT = C // P                 # 32 contraction tiles
HALF = C // 2
PRE = 6                     # update-gate gemms with split x/h contraction
NXC = 8                     # x upload chunks (2 c-tiles, 0.25 MB each)
NCH = 4                     # h upload chunks (4 c-tiles, 0.5/1 MB each)
CHT = IT // NCH             # c-tiles per h chunk
ACT_F = mybir.ActivationFunctionType

_PROGRAM = None


def _build_program():
    nc = bacc.Bacc("TRN2")

    xTd = nc.dram_tensor("xTd", [P, I // P * BS], BF16, kind="ExternalInput")
    hTb = nc.dram_tensor("hTb", [P, H // P * BS], BF16, kind="ExternalInput")
    hT32 = nc.dram_tensor("hT32", [P, H // P * BS], F32, kind="ExternalInput")
    Wu = nc.dram_tensor("Wu", [HT, P, C], BF16, kind="ExternalInput")
    Ws = nc.dram_tensor("Ws", [HT, P, C], BF16, kind="ExternalInput")
    Wp = nc.dram_tensor("Wp", [HT, P, C], BF16, kind="ExternalInput")
    bias = nc.dram_tensor("bias", [P, 3 * HT], F32, kind="ExternalInput")
    out = nc.dram_tensor("out", [HT, P, BS], F32, kind="ExternalOutput")

    CW = CHT * BS  # flat columns per upload chunk

    with tile.TileContext(nc) as tc, ExitStack() as ctx:
        singles = ctx.enter_context(tc.tile_pool(name="singles", bufs=1))
        wpool = ctx.enter_context(tc.tile_pool(name="wpool", bufs=6))
        # 6 full-bank accumulation tiles + 2 half-bank ones (last tile) = 8 banks
        pspool = ctx.enter_context(tc.tile_pool(name="ps", bufs=6, space="PSUM"))
        work = ctx.enter_context(tc.tile_pool(name="work", bufs=4))

        bias_sb = singles.tile([P, 3 * HT], F32, name="bias_sb")
        bu_sb = bias_sb[:, 0:HT]
        bs_sb = bias_sb[:, HT:2 * HT]
        bp_sb = bias_sb[:, 2 * HT:3 * HT]

        xsb = singles.tile([P, I // P * BS], BF16, name="xsb")
        hbsb = singles.tile([P, H // P * BS], BF16, name="hbsb")
        h32sb = singles.tile([P, H // P * BS], F32, name="h32sb")

        # Interleave 0.25 MB x-chunk and half-weight-block uploads 1:1 so the
        # first matmuls start after ~0.7 MB of DMA and the PE stays fed chunk
        # by chunk (subtile deps: each matmul waits only on the piece that
        # covers its columns).  `rank` records upload order for the greedy
        # matmul emission below.
        XW = 2 * BS                      # flat columns per x chunk
        wxs = [
            wpool.tile([P, HALF], BF16, tag="wx", name="wx") for _ in range(PRE)
        ]
        # two weight halves up front so each x chunk then unlocks ~0.86 us of
        # matmuls vs its ~0.72 us wire time (PE never outruns the upload)
        stream = [("w", 0, 0), ("w", 1, 0), ("x", 0), ("x", 1), ("x", 2),
                  ("w", 0, 1), ("x", 3), ("w", 1, 1), ("x", 4), ("w", 2, 0),
                  ("x", 5), ("w", 2, 1), ("x", 6), ("w", 3, 0), ("x", 7),
                  ("w", 3, 1), ("w", 4, 0), ("w", 4, 1), ("w", 5, 0),
                  ("w", 5, 1)]
        assert sorted(e for e in stream if e[0] == "x") == [
            ("x", c) for c in range(NXC)
        ]
        assert sorted((e[1], e[2]) for e in stream if e[0] == "w") == [
            (i, half) for i in range(PRE) for half in range(2)
        ]
        x_rank = {}
        wx_rank = {}
        for rank, e in enumerate(stream):
            if e[0] == "x":
                c = e[1]
                nc.sync.dma_start(xsb[:, c * XW:(c + 1) * XW],
                                  xTd[:, c * XW:(c + 1) * XW])
                x_rank[c] = rank
            else:
                _, i, half = e
                cols = slice(half * HALF // 2, (half + 1) * HALF // 2)
                nc.sync.dma_start(wxs[i][:, cols], Wu[i, :, cols])
                wx_rank[(i, half)] = rank
        for c in range(NCH):
            nc.sync.dma_start(hbsb[:, c * CW:(c + 1) * CW],
                              hTb[:, c * CW:(c + 1) * CW])
        # biases are first needed by the sigmoid at ~30 us; keep their DMA
        # out of the critical prologue stream.
        nc.sync.dma_start(bias_sb[:], bias[:])

        # combined.T views: 16 x c-tiles then 16 h c-tiles (bf16 [128, 512])
        comb = [xsb[:, n * BS:(n + 1) * BS] for n in range(IT)]
        comb += [hbsb[:, i * BS:(i + 1) * BS] for i in range(HT)]
        h32 = [h32sb[:, i * BS:(i + 1) * BS] for i in range(HT)]

        upd = [
            singles.tile([P, BS], BF16, name=f"upd{i}", tag=f"upd{i}")
            for i in range(HT)
        ]
        selb = [
            singles.tile([P, BS], BF16, name=f"selb{i}", tag=f"selb{i}")
            for i in range(HT)
        ]

        def load_w(W, i):
            wx = wpool.tile([P, HALF], BF16, tag="wx", name="wx")
            nc.sync.dma_start(wx[:], W[i, :, 0:HALF])
            wh = wpool.tile([P, HALF], BF16, tag="wh", name="wh")
            nc.sync.dma_start(wh[:], W[i, :, HALF:C])
            return wx, wh

        def mm_half(ps, w, rhs_tiles, n0, n1, start, stop, cols=None):
            for n in range(n0, n1):
                w_ap = w[:, (n - n0) * P:(n - n0 + 1) * P]
                r = rhs_tiles[n]
                nc.tensor.matmul(
                    ps,
                    w_ap,
                    r if cols is None else r[:, cols],
                    start=(start and n == n0),
                    stop=(stop and n == n1 - 1),
                )

        # ---- update gate: upd[i] = h * sigmoid(z_u) ----
        # First PRE gemms: x-half contraction, emitted as 2-matmul units in
        # upload-readiness order across the PRE PSUM banks.
        psA = []
        for i in range(PRE):
            ps = pspool.tile([P, BS], F32, tag="ps", name="ps")
            psA.append(ps)
        units = sorted(
            ((max(x_rank[c], wx_rank[(i, c // (NXC // 2))]), i, c)
             for i in range(PRE) for c in range(NXC)),
            key=lambda u: (u[0], u[2], u[1]),
        )
        started = set()
        for _, i, c in units:
            for n in (2 * c, 2 * c + 1):
                nc.tensor.matmul(
                    psA[i], wxs[i][:, n * P:(n + 1) * P], comb[n],
                    start=(i not in started), stop=False,
                )
                started.add(i)

        def finish_update(i, ps):
            u = work.tile([P, BS], BF16, tag="u", name="u")
            nc.scalar.activation(u[:], ps[:], ACT_F.Sigmoid, bias=bu_sb[:, i:i + 1])
            nc.vector.tensor_mul(upd[i][:], comb[IT + i], u[:])

        for i in range(PRE):
            wh = wpool.tile([P, HALF], BF16, tag="wh", name="wh")
            nc.sync.dma_start(wh[:], Wu[i, :, HALF:C])
            mm_half(psA[i], wh, comb, IT, CT, start=False, stop=True)
            finish_update(i, psA[i])

        for i in range(PRE, HT):
            wx, wh = load_w(Wu, i)
            ps = pspool.tile([P, BS], F32, tag="ps", name="ps")
            mm_half(ps, wx, comb, 0, IT, start=True, stop=False)
            mm_half(ps, wh, comb, IT, CT, start=False, stop=True)
            finish_update(i, ps)

        # fp32 h: only needed from the select phase on (blend terms), so its
        # upload is issued after the update-phase weight loads.
        for c in range(NCH):
            nc.sync.dma_start(h32sb[:, c * CW:(c + 1) * CW],
                              hT32[:, c * CW:(c + 1) * CW])

        # ---- select gate ----
        # sel kept bf16 (it only multiplies |tanh| <= 1 in the blend); the
        # numerically sensitive term h*(1-sel) is computed here in fp32 and
        # overwrites h32[i] in place (h itself is not needed afterwards).
        for i in range(HT):
            wx, wh = load_w(Ws, i)
            ps = pspool.tile([P, BS], F32, tag="ps", name="ps")
            mm_half(ps, wx, comb, 0, IT, start=True, stop=False)
            mm_half(ps, wh, comb, IT, CT, start=False, stop=True)
            s32 = work.tile([P, BS], F32, tag="s32", name="s32")
            nc.scalar.activation(s32[:], ps[:], ACT_F.Sigmoid, bias=bs_sb[:, i:i + 1])
            nc.vector.tensor_copy(selb[i][:], s32[:])
            nc.vector.tensor_mul(s32[:], h32[i], s32[:])
            nc.vector.tensor_sub(h32[i], h32[i], s32[:])

        hs = h32  # h32[i] now holds h * (1 - sel)

        # ---- predictions + blend: h_new = hs + sel * tanh(z_p) ----
        newcomb = comb[:IT] + [upd[i][:] for i in range(HT)]

        def blend(i, ps_ap, cols, otag):
            n = cols.stop - cols.start
            p_t = work.tile([P, n], F32, tag=f"p{otag}", name="p_t")
            nc.scalar.activation(p_t[:], ps_ap, ACT_F.Tanh, bias=bp_sb[:, i:i + 1])
            o = work.tile([P, n], F32, tag=f"o{otag}", name="o")
            nc.vector.tensor_mul(o[:], p_t[:], selb[i][:, cols])
            nc.vector.tensor_add(o[:], o[:], hs[i][:, cols])
            nc.sync.dma_start(out[i, :, cols], o[:])

        for i in range(HT - 1):
            wx, wh = load_w(Wp, i)
            ps = pspool.tile([P, BS], F32, tag="ps", name="ps")
            mm_half(ps, wx, newcomb, 0, IT, start=True, stop=False)
            mm_half(ps, wh, newcomb, IT, CT, start=False, stop=True)
            blend(i, ps[:], slice(0, BS), "f")

        # Last tile: two half-batch accumulation groups so the first half's
        # tanh+blend+store overlaps the second half's matmuls.
        i = HT - 1
        wx, wh = load_w(Wp, i)
        for hcol in range(2):
            cols = slice(hcol * (BS // 2), (hcol + 1) * (BS // 2))
            ps = pspool.tile([P, BS // 2], F32, tag="pshalf", name="pshalf",
                             bufs=2)
            mm_half(ps, wx, newcomb, 0, IT, start=True, stop=False, cols=cols)
            mm_half(ps, wh, newcomb, IT, CT, start=False, stop=True, cols=cols)
            blend(i, ps[:], cols, "h")

    nc.finalize()
    return nc


def _get_program():
    global _PROGRAM
    if _PROGRAM is None:
        _PROGRAM = _build_program()
    return _PROGRAM


def _pack_weight(w):
    """[H, C] fp32 -> [HT, P, C] bf16 with [i, p, n*128+m] = W[i*128+m, n*128+p].

    Slice [i] is then an SBUF block whose column window n*128:(n+1)*128 is the
    stationary operand (lhsT = W.T tile) for contraction tile n.
    """
    wb = np.asarray(w, dtype=np.float32).astype(NPBF16)
    return np.ascontiguousarray(
        wb.reshape(HT, P, CT, P).transpose(0, 3, 2, 1).reshape(HT, P, C)
    )


def _pack_act(a, np_dtype):
    """[BS, F] -> flat [P, F//P * BS] with [p, n*BS+b] = a[b, n*128+p]."""
    ft = a.shape[1] // P
    return np.ascontiguousarray(
        np.asarray(a, dtype=np_dtype).reshape(BS, ft, P).transpose(2, 1, 0)
        .reshape(P, ft * BS)
    )


def _prep_inputs(x, h, W_update, b_update, W_select, b_select, W_predict, b_predict):
    x = np.asarray(x, dtype=np.float32)
    h = np.asarray(h, dtype=np.float32)

    Wu = _pack_weight(W_update)
    Ws = _pack_weight(W_select)
    Wp = _pack_weight(W_predict)
    bias = np.ascontiguousarray(
        np.concatenate(
            [
                np.asarray(b, dtype=np.float32).reshape(HT, P).T
                for b in (b_update, b_select, b_predict)
            ],
            axis=1,
        )
    )

    in_maps = []
    for c in range(NCORES):
        rows = slice(c * BS, (c + 1) * BS)
        in_maps.append(
            {
                "xTd": _pack_act(x[rows], NPBF16),
                "hTb": _pack_act(h[rows], NPBF16),
                "hT32": _pack_act(h[rows], np.float32),
                "Wu": Wu,
                "Ws": Ws,
                "Wp": Wp,
                "bias": bias,
            }
        )
    return in_maps


def kernel(x, h, W_update, b_update, W_select, b_select, W_predict, b_predict,
           _trace=False):
    nc = _get_program()
    in_maps = _prep_inputs(
        x, h, W_update, b_update, W_select, b_select, W_predict, b_predict
    )
    res = run_bass_kernel_spmd(
        nc, in_maps, core_ids=list(range(NCORES)), trace=_trace
    )
    h_new = np.empty((B, H), dtype=np.float32)
    for c in range(NCORES):
        rows = slice(c * BS, (c + 1) * BS)
        h_new[rows] = res.results[c]["out"].reshape(H, BS).T
    if _trace:
        return h_new, res
    return h_new
